# revision 1
# baseline (speedup 1.0000x reference)
"""Trainium2 Bass kernel for nn_DGL_Net (3-layer GraphConv GNN, 50000 nodes, 800k edges).

Strategy (8 NeuronCores, SPMD):
  - Host: relabel nodes into 392 balanced tiles of 128 nodes (<=2046 in-edges per
    tile), 49 tiles per core. Per layer: local matmul (bf16) -> scale by c_src ->
    AllGather of the per-node activations -> per-edge dma_gather (4 SWDGE queues)
    -> one-hot (Sel) matmul aggregation in PSUM -> scale by c_dst + bias (+relu /
    log_softmax).
  - segment_sum is computed as PSUM[f,d] += G[e,f].T @ Sel[e,d] per 128-edge
    chunk, where Sel[e,d] = (dst_local[e] == d) is built on the vector engine.
    Dummy (padding) slots carry dst_local=-1 so their Sel column is all-zero.
  - int16 gather indices: gather base is offset +32768 rows so idx = row-32768
    spans the whole [0, 50176) row space within int16. The last slot of every
    1024-index gather call is a reserved dummy with idx>=0 (defeats the ucode's
    trailing-negative trim).
"""
import os
import sys

sys.path.insert(0, '/opt/trn_rl_repo')

import numpy as np
import ml_dtypes

import concourse.bass as bass
import concourse.bacc as bacc
import concourse.mybir as mybir
import concourse.tile as tile
from concourse.bass_utils import run_bass_kernel_spmd

BF16 = ml_dtypes.bfloat16

N_NODES = 50000
N_CORES = 8
TILE_N = 128                 # nodes per tile
TILES_PER_CORE = 49
N_TILES = N_CORES * TILES_PER_CORE      # 392
ROWS_PER_CORE = TILES_PER_CORE * TILE_N  # 6272
N_ROWS = N_CORES * ROWS_PER_CORE         # 50176
R_CHUNKS = 16                # edge chunks (of 128 slots) per tile
SLOTS_PER_TILE = R_CHUNKS * 128          # 2048
TILE_EDGE_CAP = SLOTS_PER_TILE - 2       # 2046 (2 reserved call-end dummies)
SLOTS = TILES_PER_CORE * SLOTS_PER_TILE  # 100352 per core
CALL = 1024                  # idxs per dma_gather call
N_CALLS = SLOTS // CALL      # 98
CHUNKS = TILES_PER_CORE * R_CHUNKS       # 784 chunks per core
IDX_OFF = 32768              # gather base offset (int16 trick)
F_IN = 1433
F_IN_P = 1536                # padded to 12*128
KC1 = F_IN_P // 128          # 12
F1 = 256
F2 = 32
F3 = 7
FPAD = 128                   # padded row width for M2/M3 gather (256B elems)

last_exec_time_ns = None


def _preprocess(edge_index):
    """Graph preprocessing: normalization constants, node->($core,tile,lane)
    relabeling with balanced per-tile in-degree, per-core edge slot tables."""
    src = np.asarray(edge_index[0], dtype=np.int64)
    dst = np.asarray(edge_index[1], dtype=np.int64)
    n_edges = src.shape[0]

    deg_out = np.bincount(src, minlength=N_NODES).astype(np.float64)
    deg_in = np.bincount(dst, minlength=N_NODES).astype(np.float64)
    c_src = (1.0 / np.sqrt(np.maximum(deg_out, 1.0))).astype(np.float32)
    c_dst = (1.0 / np.sqrt(np.maximum(deg_in, 1.0))).astype(np.float32)

    # --- greedy balanced tile packing by in-degree ---
    import heapq
    order = np.argsort(-deg_in, kind='stable')
    heap = [(0.0, 0, t) for t in range(N_TILES)]  # (load, count, tile)
    heapq.heapify(heap)
    tile_nodes = [[] for _ in range(N_TILES)]
    tile_load = np.zeros(N_TILES)
    deferred = []
    for v in order:
        dv = deg_in[v]
        while True:
            load, cnt, t = heapq.heappop(heap)
            if cnt >= TILE_N:
                continue  # stale/full
            if load + dv > TILE_EDGE_CAP:
                deferred.append((load, cnt, t))
                continue
            break
        tile_nodes[t].append(int(v))
        tile_load[t] = load + dv
        heapq.heappush(heap, (load + dv, cnt + 1, t))
        for item in deferred:
            heapq.heappush(heap, item)
        deferred = []
    assert max(tile_load) <= TILE_EDGE_CAP

    # sort tiles by load desc, group by 8, core c takes c-th of each group
    tsort = np.argsort(-tile_load, kind='stable')
    # tile_of_slot[core][k] = global tile id
    tile_assign = np.empty((N_CORES, TILES_PER_CORE), dtype=np.int64)
    for k in range(TILES_PER_CORE):
        for c in range(N_CORES):
            tile_assign[c, k] = tsort[k * N_CORES + c]

    # row mapping: row = c*ROWS_PER_CORE + k*128 + lane
    row_of_node = np.full(N_NODES, -1, dtype=np.int64)
    node_of_row = np.full(N_ROWS, -1, dtype=np.int64)  # -1 = virtual pad node
    for c in range(N_CORES):
        for k in range(TILES_PER_CORE):
            t = tile_assign[c, k]
            nodes = tile_nodes[t]
            base = c * ROWS_PER_CORE + k * TILE_N
            for lane, v in enumerate(nodes):
                row_of_node[v] = base + lane
                node_of_row[base + lane] = v
    assert (row_of_node >= 0).all()

    # --- per-core edge slot tables ---
    dst_row = row_of_node[dst]
    src_row = row_of_node[src]
    e_core = dst_row // ROWS_PER_CORE
    e_tile = (dst_row % ROWS_PER_CORE) // TILE_N   # k within core
    e_lane = dst_row % TILE_N

    idx_flat = np.zeros((N_CORES, SLOTS), dtype=np.int16)      # pad idx = 0
    dst_flat = np.full((N_CORES, SLOTS), -1, dtype=np.int16)   # pad dst = -1

    # group edges by (core, tile) and assign slot positions
    key = e_core * TILES_PER_CORE + e_tile
    eorder = np.argsort(key, kind='stable')
    key_s = key[eorder]
    # position within group
    grp_start = np.searchsorted(key_s, np.arange(N_CORES * TILES_PER_CORE))
    pos_in_grp = np.arange(n_edges) - grp_start[key_s]
    # slot within tile: skip reserved slots 1023 and 2047
    j = pos_in_grp
    slot_in_tile = j + (j >= 1023).astype(np.int64) # j>=1023 shifts past slot 1023
    assert slot_in_tile.max() < SLOTS_PER_TILE - 1  # never hits 2047
    slots_abs = key_s % TILES_PER_CORE * SLOTS_PER_TILE + slot_in_tile
    cores_s = key_s // TILES_PER_CORE
    idx_flat[cores_s, slots_abs] = (src_row[eorder] - IDX_OFF).astype(np.int16)
    dst_flat[cores_s, slots_abs] = e_lane[eorder].astype(np.int16)

    # wrap idx to [128, SLOTS/16] (idx i -> [i%16 replicated, i//16])
    cols = SLOTS // 16
    idx_tile = np.zeros((N_CORES, 128, cols), dtype=np.int16)
    for c in range(N_CORES):
        w = idx_flat[c].reshape(cols, 16).T  # [16, cols]
        idx_tile[c] = np.tile(w, (8, 1))
    # one-hot Sel cache per chunk: [128e, CHUNKS*128d] bf16
    selc = np.zeros((N_CORES, 128, CHUNKS * 128), dtype=BF16)
    dr = np.arange(128, dtype=np.int16)
    for c in range(N_CORES):
        dd = dst_flat[c].reshape(CHUNKS, 128)  # [ch, e]
        oh = (dd[:, :, None] == dr[None, None, :])  # [ch, e, d]
        selc[c] = oh.transpose(1, 0, 2).reshape(128, CHUNKS * 128).astype(BF16)

    # per-core normalization tables
    cd_row = np.where(node_of_row >= 0, c_dst[np.maximum(node_of_row, 0)], 1.0)
    cs_row = np.where(node_of_row >= 0, c_src[np.maximum(node_of_row, 0)], 1.0)
    cd_core = cd_row.reshape(N_CORES, ROWS_PER_CORE).astype(np.float32)
    cs_core = cs_row.reshape(N_CORES, ROWS_PER_CORE).astype(np.float32)
    cdst_rep = np.repeat(cd_core[:, None, :], 128, axis=1)  # [C,128,6272]
    cdst_pp = cd_core.reshape(N_CORES, TILES_PER_CORE, 128).transpose(0, 2, 1).copy()
    csrc_t = cs_core.reshape(N_CORES, TILES_PER_CORE, 128).transpose(0, 2, 1).copy()

    return dict(row_of_node=row_of_node, node_of_row=node_of_row,
                idx_tile=idx_tile, selc=selc,
                cdst_rep=cdst_rep.astype(np.float32), cdst_pp=cdst_pp,
                csrc_t=csrc_t)


def _build_nc():
    nc = bacc.Bacc("TRN2", target_bir_lowering=False, debug=False,
                   enable_asserts=True, num_devices=N_CORES, num_swdge_queues=4)
    dt = mybir.dt
    inp = {}
    inp['xT'] = nc.dram_tensor("xT", [F_IN_P, ROWS_PER_CORE], dt.bfloat16, kind="ExternalInput")
    inp['W1'] = nc.dram_tensor("W1", [F_IN_P, F1], dt.bfloat16, kind="ExternalInput")
    inp['W2'] = nc.dram_tensor("W2", [F1, F2], dt.bfloat16, kind="ExternalInput")
    inp['W3'] = nc.dram_tensor("W3", [F2, F3], dt.bfloat16, kind="ExternalInput")
    inp['idx'] = nc.dram_tensor("idx", [128, SLOTS // 16], dt.int16, kind="ExternalInput")
    inp['selc'] = nc.dram_tensor("selc", [128, CHUNKS * 128], dt.bfloat16, kind="ExternalInput")
    inp['cdst_rep'] = nc.dram_tensor("cdst_rep", [128, ROWS_PER_CORE], dt.float32, kind="ExternalInput")
    inp['cdst_pp'] = nc.dram_tensor("cdst_pp", [128, TILES_PER_CORE], dt.float32, kind="ExternalInput")
    inp['csrc_t'] = nc.dram_tensor("csrc_t", [128, TILES_PER_CORE], dt.float32, kind="ExternalInput")
    inp['b1pp'] = nc.dram_tensor("b1pp", [128, 2], dt.float32, kind="ExternalInput")
    inp['b2pp'] = nc.dram_tensor("b2pp", [128, 1], dt.float32, kind="ExternalInput")
    inp['b3t'] = nc.dram_tensor("b3t", [128, F3], dt.float32, kind="ExternalInput")
    out_t = nc.dram_tensor("out", [ROWS_PER_CORE, F3], dt.float32, kind="ExternalOutput")

    m1_own = nc.dram_tensor("m1_own", [ROWS_PER_CORE, F1], dt.bfloat16)
    m1_full = nc.dram_tensor("m1_full", [N_ROWS, F1], dt.bfloat16, addr_space="Shared")
    m2_own = nc.dram_tensor("m2_own", [ROWS_PER_CORE, FPAD], dt.bfloat16)
    m2_full = nc.dram_tensor("m2_full", [N_ROWS, FPAD], dt.bfloat16, addr_space="Shared")
    m3_own = nc.dram_tensor("m3_own", [ROWS_PER_CORE, FPAD], dt.bfloat16)
    m3_full = nc.dram_tensor("m3_full", [N_ROWS, FPAD], dt.bfloat16, addr_space="Shared")

    AL = mybir.AluOpType
    AF = mybir.ActivationFunctionType
    RG = [list(range(N_CORES))]

    with tile.TileContext(nc) as tc:
        with tc.tile_pool(name="const", bufs=1) as constp, \
             tc.tile_pool(name="big", bufs=1) as bigp, \
             tc.tile_pool(name="xstream", bufs=2) as xp, \
             tc.tile_pool(name="work", bufs=3) as wp, \
             tc.tile_pool(name="gpool", bufs=4) as gp, \
             tc.tile_pool(name="selp", bufs=6) as selp, \
             tc.tile_pool(name="psA", bufs=4, space="PSUM") as psA, \
             tc.tile_pool(name="psB", bufs=2, space="PSUM") as psB, \
             tc.tile_pool(name="psmm", bufs=2, space="PSUM") as psmm:

            # ---- resident constants ----
            w1_t = constp.tile([128, KC1, F1], mybir.dt.bfloat16)
            nc.sync.dma_start(w1_t[:], inp['W1'].rearrange("(kc p) n -> p kc n", p=128))
            w2_t = constp.tile([128, 2, F2], mybir.dt.bfloat16)
            nc.sync.dma_start(w2_t[:], inp['W2'].rearrange("(kc p) n -> p kc n", p=128))
            w3_t = constp.tile([F2, F3], mybir.dt.bfloat16)
            nc.sync.dma_start(w3_t[:], inp['W3'][:, :])
            idx_t = constp.tile([128, SLOTS // 16], mybir.dt.int16)
            nc.sync.dma_start(idx_t[:], inp['idx'][:, :])

            cdrep_t = constp.tile([128, ROWS_PER_CORE], mybir.dt.float32)
            nc.sync.dma_start(cdrep_t[:], inp['cdst_rep'][:, :])
            cdpp_t = constp.tile([128, TILES_PER_CORE], mybir.dt.float32)
            nc.sync.dma_start(cdpp_t[:], inp['cdst_pp'][:, :])
            cs_t = constp.tile([128, TILES_PER_CORE], mybir.dt.float32)
            nc.sync.dma_start(cs_t[:], inp['csrc_t'][:, :])
            b1_t = constp.tile([128, 2], mybir.dt.float32)
            nc.sync.dma_start(b1_t[:], inp['b1pp'][:, :])
            b2_t = constp.tile([128, 1], mybir.dt.float32)
            nc.sync.dma_start(b2_t[:], inp['b2pp'][:, :])
            b3_t = constp.tile([128, F3], mybir.dt.float32)
            nc.sync.dma_start(b3_t[:], inp['b3t'][:, :])


            h1t = bigp.tile([128, 2, ROWS_PER_CORE], mybir.dt.bfloat16)  # H1.T
            h2t = bigp.tile([F2, ROWS_PER_CORE], mybir.dt.bfloat16)      # H2.T
            xall = bigp.tile([128, TILES_PER_CORE * F3], mybir.dt.float32)  # logits

            # ---- phase 1: M1 = (X @ W1) * c_src ----
            blocks = [(i * 512, 512) for i in range(12)] + [(6144, 128)]
            for c0, bs in blocks:
                xt = xp.tile([128, KC1, bs], mybir.dt.bfloat16, tag="xt")
                nc.sync.dma_start(
                    xt[:, :, :bs],
                    inp['xT'][:, c0:c0 + bs].rearrange("(kc p) n -> p kc n", p=128))
                for sub in range(bs // 128):
                    t_idx = (c0 + sub * 128) // 128
                    ps = psmm.tile([128, F1], mybir.dt.float32, tag="mm1")
                    for kc in range(KC1):
                        nc.tensor.matmul(ps[:], xt[:, kc, sub * 128:(sub + 1) * 128],
                                         w1_t[:, kc, :], start=(kc == 0), stop=(kc == KC1 - 1))
                    ob = wp.tile([128, F1], mybir.dt.bfloat16, tag="m1o")
                    nc.vector.tensor_scalar(ob[:], ps[:], cs_t[:, t_idx:t_idx + 1], None, AL.mult)
                    nc.sync.dma_start(m1_own[t_idx * 128:(t_idx + 1) * 128, :], ob[:])

            nc.gpsimd.collective_compute("AllGather", AL.bypass, replica_groups=RG,
                                         ins=[m1_own[:, :]], outs=[m1_full[:, :]])

            # ---- agg helper ----
            def agg_layer(m_full, elem, consume_chunk, finish_tile):
                cur = {}
                for call in range(N_CALLS):
                    g = gp.tile([128, CALL // 128, elem], mybir.dt.bfloat16, tag=f"g{elem}")
                    nc.gpsimd.dma_gather(
                        g[:], m_full[IDX_OFF:, :],
                        idx_t[:, call * (CALL // 16):(call + 1) * (CALL // 16)],
                        CALL, CALL, elem, queue_num=call % 4)
                    selg = selp.tile([128, CALL], mybir.dt.bfloat16, tag="selg", name="selg")
                    nc.sync.dma_start(selg[:], inp['selc'][:, call * CALL:(call + 1) * CALL])
                    for j in range(CALL // 128):
                        ch = call * (CALL // 128) + j
                        t_idx = ch // R_CHUNKS
                        first = (ch % R_CHUNKS == 0)
                        last = (ch % R_CHUNKS == R_CHUNKS - 1)
                        sel = selg[:, j * 128:(j + 1) * 128]
                        consume_chunk(cur, g, j, sel, t_idx, first, last)
                        if last:
                            finish_tile(cur, t_idx)
                            cur.clear()

            # ---- layer 1 aggregation -> H1T ----
            def l1_chunk(cur, g, j, sel, t_idx, first, last):
                if first:
                    cur[0] = psA.tile([128, 128], mybir.dt.float32, tag="aggA", name="psa1")
                    cur[1] = psB.tile([128, 128], mybir.dt.float32, tag="aggB", name="psb1")
                for fc in range(2):
                    nc.tensor.matmul(cur[fc][:], g[:, j, fc * 128:(fc + 1) * 128],
                                     sel, start=first, stop=last)

            def l1_tile(cur, t_idx):
                sl = slice(t_idx * 128, (t_idx + 1) * 128)
                for fc in range(2):
                    nc.vector.tensor_tensor(h1t[:, fc, sl], cur[fc][:],
                                            cdrep_t[:, sl], AL.mult)
                    nc.scalar.activation(h1t[:, fc, sl], h1t[:, fc, sl],
                                         AF.Relu, bias=b1_t[:, fc:fc + 1])

            agg_layer(m1_full, F1, l1_chunk, l1_tile)

            # ---- phase 2: M2 = (H1 @ W2) * c_src ----
            for t_idx in range(TILES_PER_CORE):
                sl = slice(t_idx * 128, (t_idx + 1) * 128)
                ps = psmm.tile([128, F2], mybir.dt.float32, tag="mm1")
                for fc in range(2):
                    nc.tensor.matmul(ps[:], h1t[:, fc, sl], w2_t[:, fc, :],
                                     start=(fc == 0), stop=(fc == 1))
                ob = wp.tile([128, FPAD], mybir.dt.bfloat16, tag="m2o")
                nc.vector.tensor_scalar(ob[:, 0:F2], ps[:], cs_t[:, t_idx:t_idx + 1], None, AL.mult)
                nc.sync.dma_start(m2_own[t_idx * 128:(t_idx + 1) * 128, :], ob[:])

            nc.gpsimd.collective_compute("AllGather", AL.bypass, replica_groups=RG,
                                         ins=[m2_own[:, :]], outs=[m2_full[:, :]])

            # ---- layer 2 aggregation -> H2T ----
            def l2_chunk(cur, g, j, sel, t_idx, first, last):
                if first:
                    cur[0] = psA.tile([F2, 128], mybir.dt.float32, tag="aggA", name="psa2")
                nc.tensor.matmul(cur[0][:], g[:, j, 0:F2], sel, start=first, stop=last)

            def l2_tile(cur, t_idx):
                sl = slice(t_idx * 128, (t_idx + 1) * 128)
                nc.vector.tensor_tensor(h2t[:, sl], cur[0][:], cdrep_t[0:F2, sl], AL.mult)
                nc.scalar.activation(h2t[:, sl], h2t[:, sl], AF.Relu, bias=b2_t[0:F2, 0:1])

            agg_layer(m2_full, FPAD, l2_chunk, l2_tile)

            # ---- phase 3: M3 = (H2 @ W3) * c_src ----
            for t_idx in range(TILES_PER_CORE):
                sl = slice(t_idx * 128, (t_idx + 1) * 128)
                ps = psmm.tile([128, F3], mybir.dt.float32, tag="mm1")
                nc.tensor.matmul(ps[:], h2t[:, sl], w3_t[:], start=True, stop=True)
                ob = wp.tile([128, FPAD], mybir.dt.bfloat16, tag="m3o")
                nc.vector.tensor_scalar(ob[:, 0:F3], ps[:], cs_t[:, t_idx:t_idx + 1], None, AL.mult)
                nc.sync.dma_start(m3_own[t_idx * 128:(t_idx + 1) * 128, :], ob[:])

            nc.gpsimd.collective_compute("AllGather", AL.bypass, replica_groups=RG,
                                         ins=[m3_own[:, :]], outs=[m3_full[:, :]])

            # ---- layer 3 aggregation -> log_softmax -> out ----
            def l3_chunk(cur, g, j, sel, t_idx, first, last):
                if first:
                    cur[0] = psA.tile([128, F3], mybir.dt.float32, tag="aggA", name="psa3")
                nc.tensor.matmul(cur[0][:], sel, g[:, j, 0:F3], start=first, stop=last)

            def l3_tile(cur, t_idx):
                sl3 = slice(t_idx * F3, (t_idx + 1) * F3)
                nc.vector.tensor_scalar(xall[:, sl3], cur[0][:], cdpp_t[:, t_idx:t_idx + 1], None, AL.mult)
                nc.vector.tensor_tensor(xall[:, sl3], xall[:, sl3], b3_t[:], AL.add)

            agg_layer(m3_full, FPAD, l3_chunk, l3_tile)

            exall = bigp.tile([128, TILES_PER_CORE * F3], mybir.dt.float32)
            nc.scalar.activation(exall[:], xall[:], AF.Exp)
            smv = bigp.tile([128, TILES_PER_CORE], mybir.dt.float32)
            nc.vector.tensor_reduce(
                smv[:], exall[:].rearrange("p (t f) -> p t f", f=F3),
                mybir.AxisListType.X, AL.add)
            rsv = bigp.tile([128, TILES_PER_CORE], mybir.dt.float32)
            nc.vector.reciprocal(rsv[:], smv[:])
            nlog = bigp.tile([128, TILES_PER_CORE], mybir.dt.float32)
            nc.scalar.activation(nlog[:], rsv[:], AF.Ln)
            for t_idx in range(TILES_PER_CORE):
                sl3 = slice(t_idx * F3, (t_idx + 1) * F3)
                ox = wp.tile([128, F3], mybir.dt.float32, tag="ox", name="ox")
                nc.vector.tensor_scalar(ox[:], xall[:, sl3], nlog[:, t_idx:t_idx + 1], None, AL.add)
                nc.sync.dma_start(out_t[t_idx * 128:(t_idx + 1) * 128, :], ox[:])

    nc.compile()
    return nc


def _install_profile_shim():
    """Provide the missing antenv.axon_hooks module so trace=True works under axon."""
    try:
        import types
        import antenv
        if 'antenv.axon_hooks' in sys.modules:
            return
        _hook = [None]
        mod = types.ModuleType('antenv.axon_hooks')
        mod.set_axon_ntff_profile_hook = lambda h: _hook.__setitem__(0, h)
        mod.get_axon_ntff_profile_hook = lambda: _hook[0]
        sys.modules['antenv.axon_hooks'] = mod
        antenv.axon_hooks = mod
        from trn_agent_boot.trn_boot import _ntff_profile_via_ctypes
        mod.set_axon_ntff_profile_hook(
            _ntff_profile_via_ctypes('/opt/axon/libaxon_pjrt.so'))
    except Exception:
        pass


_CACHE = {}


def kernel(features, edge_index, W1, b1, W2, b2, W3, b3):
    global last_exec_time_ns
    features = np.asarray(features, dtype=np.float32)
    pre = _preprocess(np.asarray(edge_index))

    if 'nc' not in _CACHE:
        _CACHE['nc'] = _build_nc()
    nc = _CACHE['nc']

    # host-side input prep
    W1p = np.zeros((F_IN_P, F1), dtype=BF16)
    W1p[:F_IN] = np.asarray(W1, dtype=BF16)
    W2b = np.asarray(W2, dtype=BF16)
    W3b = np.asarray(W3, dtype=BF16)
    b1pp = np.asarray(b1, dtype=np.float32).reshape(2, 128).T.copy()
    b2pp = np.zeros((128, 1), dtype=np.float32)
    b2pp[:F2, 0] = np.asarray(b2, dtype=np.float32)
    b3t = np.tile(np.asarray(b3, dtype=np.float32), (128, 1))

    # features, permuted and transposed per core: [F_IN_P, 6272] bf16
    feat_b = features.astype(BF16)
    in_maps = []
    for c in range(N_CORES):
        rows = pre['node_of_row'][c * ROWS_PER_CORE:(c + 1) * ROWS_PER_CORE]
        xTc = np.zeros((F_IN_P, ROWS_PER_CORE), dtype=BF16)
        real = rows >= 0
        xTc[:F_IN, real] = feat_b[rows[real]].T
        in_maps.append({
            'xT': xTc, 'W1': W1p, 'W2': W2b, 'W3': W3b,
            'idx': pre['idx_tile'][c], 'selc': pre['selc'][c],
            'cdst_rep': pre['cdst_rep'][c], 'cdst_pp': pre['cdst_pp'][c],
            'csrc_t': pre['csrc_t'][c],
            'b1pp': b1pp, 'b2pp': b2pp, 'b3t': b3t,
        })

    trace = os.environ.get('BASS_KERNEL_TRACE', '0') == '1'
    if trace:
        _install_profile_shim()
    res = run_bass_kernel_spmd(nc, in_maps, core_ids=list(range(N_CORES)), trace=trace)
    last_exec_time_ns = res.exec_time_ns

    # assemble + inverse permute
    out_rows = np.concatenate([res.results[c]['out'] for c in range(N_CORES)], axis=0)
    out = np.empty((N_NODES, F3), dtype=np.float32)
    real = pre['node_of_row'] >= 0
    out[pre['node_of_row'][real]] = out_rows[real]
    return out

